# revision 6
# baseline (speedup 1.0000x reference)
"""MoE kernel for Trainium2 (8 NeuronCores, data-parallel over tokens).

Problem: N=8192 tokens, D=2048, E=8 experts, top-2 routing.
  gate_logits = x @ Wg; top-2 softmax -> coeff [N, E] (0 for unrouted)
  out = sum_e coeff[:, e:e+1] * (x @ We[e] + be[e])

Strategy (phase 1, dense): shard tokens across 8 cores (1024 each),
replicate weights. Per core:
  - PE-transpose x -> xT (contraction dim on partitions)
  - gating matmuls + top-2 softmax coeffs on DVE/ACT
  - bias term via coeffT @ be matmul
  - dense expert matmuls in fp32r, per-expert coeff scaling on DVE
"""

import os
import sys

# Make the NTFF profile hook importable under BASS_TRACE (the agent image's
# antenv may lack axon_hooks; bass_utils imports it when tracing).
try:
    import antenv.axon_hooks  # noqa: F401
except ImportError:
    import types

    _m = types.ModuleType("antenv.axon_hooks")
    _m._hook = None

    def _set(hook):
        _m._hook = hook

    def _get():
        return _m._hook

    _m.set_axon_ntff_profile_hook = _set
    _m.get_axon_ntff_profile_hook = _get
    sys.modules["antenv.axon_hooks"] = _m

import numpy as np

import concourse.bacc as bacc
import concourse.mybir as mybir
import concourse.tile as tile
from concourse.bass_utils import run_bass_kernel_spmd
from concourse.masks import make_identity

N, D, E = 8192, 2048, 8
NCORES = 8
NLOC = N // NCORES  # tokens per core
P = 128
C = NLOC // P       # token chunks per core
KC = D // P         # contraction chunks
NBS = 512           # free-dim block (one PSUM bank)
NB = D // NBS       # output column blocks

f32 = mybir.dt.float32
f32r = mybir.dt.float32r
Alu = mybir.AluOpType
Act = mybir.ActivationFunctionType
Axis = mybir.AxisListType

LAST_RESULT = None


def _build():
    nc = bacc.Bacc("TRN2", target_bir_lowering=False, debug=False,
                   num_devices=NCORES)
    x = nc.dram_tensor("x", [NLOC, D], f32, kind="ExternalInput").ap()
    Wg = nc.dram_tensor("Wg", [D, E], f32, kind="ExternalInput").ap()
    We = nc.dram_tensor("We", [E, D, D], f32, kind="ExternalInput").ap()
    be = nc.dram_tensor("be", [E, D], f32, kind="ExternalInput").ap()
    out = nc.dram_tensor("out", [NLOC, D], f32, kind="ExternalOutput").ap()

    with tile.TileContext(nc) as tc:
        with (
            tc.tile_pool(name="big", bufs=1) as big,
            tc.tile_pool(name="work", bufs=3) as work,
            tc.tile_pool(name="wpool", bufs=8) as wpool,
            tc.tile_pool(name="opool", bufs=8) as opool,
            tc.tile_pool(name="psum", bufs=8, space="PSUM") as psum,
        ):
            ident = big.tile([P, P], f32)
            make_identity(nc, ident[:])

            # gating weights in exact f32: routing decisions (top-2 set
            # membership) must match the fp32 reference; min top2/top3
            # logit gap on these inputs is ~1.5e-5, far above fp32 matmul
            # error but below fp32r error.
            wg_sb = big.tile([P, KC, E], f32)
            for kc in range(KC):
                nc.sync.dma_start(out=wg_sb[:, kc, :],
                                  in_=Wg[kc * P:(kc + 1) * P, :])

            be_sb = big.tile([E, D], f32r)
            nc.sync.dma_start(out=be_sb[:], in_=be[:, :].bitcast(f32r))

            xT = big.tile([P, KC, NLOC], f32r)
            coeff = big.tile([P, C, E], f32)
            coeffT = big.tile([E, NLOC], f32r)

            for c in range(C):
                cs = slice(c * P, (c + 1) * P)
                x_sb = work.tile([P, D], f32, tag="xin")
                nc.sync.dma_start(out=x_sb[:], in_=x[cs, :])
                xTg = work.tile([P, KC, P], f32, tag="xtg")
                for kc in range(KC):
                    pt = psum.tile([P, P], f32, tag="ps")
                    nc.tensor.transpose(out=pt[:],
                                        in_=x_sb[:, kc * P:(kc + 1) * P],
                                        identity=ident[:])
                    nc.vector.tensor_copy(out=xTg[:, kc, :], in_=pt[:])
                    nc.scalar.copy(out=xT[:, kc, cs], in_=pt[:])

                # gating logits for this chunk (exact f32 matmul)
                pg = psum.tile([P, E], f32, tag="ps")
                for kc in range(KC):
                    nc.tensor.matmul(pg[:],
                                     lhsT=xTg[:, kc, :],
                                     rhs=wg_sb[:, kc, :],
                                     start=(kc == 0), stop=(kc == KC - 1))
                g = work.tile([P, E], f32, tag="g")
                nc.vector.tensor_copy(out=g[:], in_=pg[:])

                # top-2 softmax -> coeff (dense [P, E], zero for unrouted)
                m1n = work.tile([P, 1], f32, tag="m1n")
                nc.vector.tensor_reduce(out=m1n[:], in_=g[:], axis=Axis.X,
                                        op=Alu.max, negate=True)
                ge1 = work.tile([P, E], f32, tag="ge1")
                nc.vector.tensor_scalar(out=ge1[:], in0=g[:],
                                        scalar1=m1n[:, 0:1], scalar2=0.0,
                                        op0=Alu.add, op1=Alu.is_ge)
                g2 = work.tile([P, E], f32, tag="g2")
                nc.vector.scalar_tensor_tensor(out=g2[:], in0=ge1[:],
                                               scalar=-1e30, in1=g[:],
                                               op0=Alu.mult, op1=Alu.add)
                m2n = work.tile([P, 1], f32, tag="m2n")
                nc.vector.tensor_reduce(out=m2n[:], in_=g2[:], axis=Axis.X,
                                        op=Alu.max, negate=True)
                mask2 = work.tile([P, E], f32, tag="mask2")
                nc.vector.tensor_scalar(out=mask2[:], in0=g[:],
                                        scalar1=m2n[:, 0:1], scalar2=0.0,
                                        op0=Alu.add, op1=Alu.is_ge)
                ex = work.tile([P, E], f32, tag="ex")
                nc.scalar.activation(out=ex[:], in_=g[:], func=Act.Exp,
                                     bias=m1n[:, 0:1], scale=1.0)
                masked = work.tile([P, E], f32, tag="masked")
                nc.vector.tensor_tensor(out=masked[:], in0=ex[:],
                                        in1=mask2[:], op=Alu.mult)
                z = work.tile([P, 1], f32, tag="z")
                nc.vector.tensor_reduce(out=z[:], in_=masked[:], axis=Axis.X,
                                        op=Alu.add)
                rz = work.tile([P, 1], f32, tag="rz")
                nc.vector.reciprocal(out=rz[:], in_=z[:])
                nc.vector.tensor_scalar_mul(out=coeff[:, c, :], in0=masked[:],
                                            scalar1=rz[:, 0:1])

                # coeffT chunk for the bias matmul
                pct = psum.tile([E, P], f32, tag="ps")
                nc.tensor.transpose(out=pct[:], in_=coeff[:, c, :],
                                    identity=ident[:])
                nc.vector.tensor_copy(out=coeffT[:, cs], in_=pct[:])

            # main expert loops
            for nb in range(NB):
                ns = slice(nb * NBS, (nb + 1) * NBS)
                osb = []
                for c in range(C):
                    cs = slice(c * P, (c + 1) * P)
                    pb = psum.tile([P, NBS], f32, tag="ps")
                    nc.tensor.matmul(pb[:],
                                     lhsT=coeffT[:, cs],
                                     rhs=be_sb[:, ns],
                                     start=True, stop=True)
                    o = opool.tile([P, NBS], f32, tag="o", name=f"o_{nb}_{c}")
                    nc.scalar.copy(out=o[:], in_=pb[:])
                    osb.append(o)
                for e in range(E):
                    ps = []
                    for c in range(C):
                        ps.append(psum.tile([P, NBS], f32, tag="ps",
                                            name=f"ps_{nb}_{e}_{c}"))
                    for kc in range(KC):
                        wt = wpool.tile([P, NBS], f32r, tag="w")
                        nc.sync.dma_start(
                            out=wt[:],
                            in_=We[e, kc * P:(kc + 1) * P, ns].bitcast(f32r))
                        for c in range(C):
                            cs = slice(c * P, (c + 1) * P)
                            nc.tensor.matmul(ps[c][:],
                                             lhsT=xT[:, kc, cs],
                                             rhs=wt[:],
                                             start=(kc == 0),
                                             stop=(kc == KC - 1))
                    for c in range(C):
                        nc.vector.scalar_tensor_tensor(
                            out=osb[c][:], in0=ps[c][:],
                            scalar=coeff[:, c, e:e + 1], in1=osb[c][:],
                            op0=Alu.mult, op1=Alu.add)
                for c in range(C):
                    nc.sync.dma_start(out=out[c * P:(c + 1) * P, ns],
                                      in_=osb[c][:])

    nc.compile()
    return nc


_NC_CACHE = None


def kernel(inputs: np.ndarray, Wg: np.ndarray, We: np.ndarray,
           be: np.ndarray) -> np.ndarray:
    global LAST_RESULT, _NC_CACHE
    inputs = np.ascontiguousarray(inputs, dtype=np.float32)
    Wg = np.ascontiguousarray(Wg, dtype=np.float32)
    We = np.ascontiguousarray(We, dtype=np.float32)
    be = np.ascontiguousarray(be, dtype=np.float32)

    if _NC_CACHE is None:
        _NC_CACHE = _build()
    nc = _NC_CACHE

    in_maps = []
    for i in range(NCORES):
        in_maps.append({
            "x": inputs[i * NLOC:(i + 1) * NLOC],
            "Wg": Wg,
            "We": We,
            "be": be,
        })
    res = run_bass_kernel_spmd(nc, in_maps, core_ids=list(range(NCORES)))
    LAST_RESULT = res
    out = np.concatenate([res.results[i]["out"] for i in range(NCORES)],
                         axis=0)
    return out
